# revision 9
# baseline (speedup 1.0000x reference)
"""Bidirectional ReGU layer on 8 Trainium2 NeuronCores.

Problem: T=512, B=64, I=H=512.
  gates = sigmoid(x@Wih^T + h@Whh^T + b); f,o = split(gates)
  c = f*c + (1-f)*tanh(x@Wc^T);  h = o*tanh(c) + (1-o)*x
Forward scan + reverse scan, outputs concatenated on feature dim.

Sharding: core = (direction, batch quarter).  Cores 0-3 run the forward
scan on batch slices of 16, cores 4-7 the reverse scan (same SPMD program;
direction is carried entirely by the per-core input data: weights and a
time-reversed input tensor).  No collectives.

Layout: everything feature-on-partition ("transposed").  The recurrent
matmul is mapping-B: stationary Whh^T tiles (bf16 -> fast weight load),
moving operand h^T [128, 16].  Input projections are batched over 32
timesteps so their matmuls get N=512.  PSUM accumulates fp32; cell state c
stays fp32; h / staged projections are bf16 (validated: rel err ~3e-3).
"""

import json
import os

import numpy as np
import ml_dtypes

import concourse.bass as bass
import concourse.mybir as mybir
from concourse.tile import TileContext
from concourse.bass_utils import run_bass_kernel_spmd

AF = mybir.ActivationFunctionType
BF16 = mybir.dt.bfloat16
F32 = mybir.dt.float32
NP_BF16 = ml_dtypes.bfloat16

B, I, H = 64, 512, 512
NCORES = 8
BC = B // 4          # batch per core (4 quarters per direction)
TCH = 32             # timesteps per projection chunk
NCH = int(os.environ.get("REGU_NCH", "16"))  # chunks (16 => T=512)
T = NCH * TCH
KO = I // 128        # contraction k-tiles (4)
MG = 2 * H // 128    # gate feature tiles (8)
MC = H // 128        # cell feature tiles (4)

# ---------------------------------------------------------------------------
# Walrus workaround: this container's walrus rejects control instructions
# (Drain etc.) carrying more than ~2 semaphore waits.  Split excess waits
# onto single-wait EventSemaphore instructions on the same engine, same
# program position (semantics preserved).
# ---------------------------------------------------------------------------
_CTRL_OPS = {"Drain", "NoOp", "Halt", "EventSemaphore"}
_uid = [0]


def _split_ctrl_waits(bir_bytes, max_waits=1):
    bir = json.loads(bir_bytes)
    for fn in bir.get("functions", []):
        for blk in fn.get("blocks", []):
            out = []
            for inst in blk.get("instructions", []):
                si = inst.get("sync_info")
                waits = (si or {}).get("on_wait") or []
                cap = 1 if inst.get("opcode") in _CTRL_OPS else max_waits
                if si is not None and len(waits) > cap:
                    keep = waits[-cap:] if cap else []
                    for w in waits[: len(waits) - len(keep)]:
                        _uid[0] += 1
                        out.append({
                            "debug": inst.get("debug", 0),
                            "engine": inst["engine"],
                            "ins": [], "outs": [],
                            "name": f"I-wfix-{_uid[0]}",
                            "opcode": "EventSemaphore",
                            "sync_info": {"on_update": [], "on_wait": [w]},
                        })
                    inst["sync_info"] = {"on_update": si.get("on_update") or [], "on_wait": keep}
                out.append(inst)
            blk["instructions"] = out
    return json.dumps(bir).encode()


def _install_waitfix(nc):
    orig = nc.to_json_bytes
    nc.to_json_bytes = lambda: _split_ctrl_waits(orig())
    return nc


# ---------------------------------------------------------------------------
# Kernel graph (one SPMD program for all 8 cores)
# ---------------------------------------------------------------------------

def _build():
    nc = bass.Bass()

    # inputs (per core): time-major-chunked transposed x, transposed weights
    xT = nc.dram_tensor("xT", [NCH, I, TCH, BC], BF16, kind="ExternalInput")
    wih = nc.dram_tensor("wih", [I, 2 * H], BF16, kind="ExternalInput")   # Wih^T
    whh = nc.dram_tensor("whh", [H, 2 * H], BF16, kind="ExternalInput")   # Whh^T
    wc = nc.dram_tensor("wc", [I, H], BF16, kind="ExternalInput")         # Wc^T
    bias = nc.dram_tensor("bias", [MG, 128], F32, kind="ExternalInput")
    hT0 = nc.dram_tensor("hT0", [H, BC], BF16, kind="ExternalInput")
    cT0 = nc.dram_tensor("cT0", [H, BC], F32, kind="ExternalInput")

    # outputs: h for every step (bf16, feature-tiled), final c (fp32)
    outD = nc.dram_tensor("outD", [MC, 128, T, BC], BF16, kind="ExternalOutput")
    cfin = nc.dram_tensor("cfin", [128, MC, BC], F32, kind="ExternalOutput")

    with TileContext(nc) as tc:
        with (
            tc.tile_pool(name="const", bufs=1) as const,
            tc.tile_pool(name="stream", bufs=2) as stream,
            tc.tile_pool(name="outp", bufs=2) as outp,
            tc.tile_pool(name="temp", bufs=3) as temp,
            tc.tile_pool(name="psum_p", bufs=2, space="PSUM") as psum_p,
            tc.tile_pool(name="psum_r", bufs=2, space="PSUM") as psum_r,
        ):
            # ---- constants / state ----
            wih_sb = const.tile([128, KO, 2 * H], BF16, tag="wih")
            nc.sync.dma_start(wih_sb[:], wih.rearrange("(ko p) m -> p ko m", p=128))
            whh_sb = const.tile([128, KO, 2 * H], BF16, tag="whh")
            nc.sync.dma_start(whh_sb[:], whh.rearrange("(ko p) m -> p ko m", p=128))
            wc_sb = const.tile([128, KO, H], BF16, tag="wc")
            nc.sync.dma_start(wc_sb[:], wc.rearrange("(ko p) m -> p ko m", p=128))
            bias_sb = const.tile([128, MG], F32, tag="bias")
            nc.sync.dma_start(bias_sb[:], bias.rearrange("m p -> p m"))
            c_sb = const.tile([128, MC, BC], F32, tag="cstate")
            nc.sync.dma_start(c_sb[:], cT0.rearrange("(k p) b -> p k b", p=128))

            def make_proj(ch):
                """Chunk ch projection emitters, split into 13 pieces so they
                can be interleaved into the previous chunk's recurrence."""
                x_sb = stream.tile([128, KO, TCH, BC], BF16, tag="xch")
                xg_sb = stream.tile([128, MG, TCH, BC], BF16, tag="xg")
                txc_sb = stream.tile([128, MC, TCH, BC], BF16, tag="txc")

                def dma_piece():
                    nc.sync.dma_start(
                        x_sb[:], xT[ch].rearrange("(ko p) t b -> p ko t b", p=128))

                def mm_piece(w_sb, m, out_sb, act, abias):
                    def run():
                        ps = psum_p.tile([128, TCH, BC], F32, tag="pp")
                        for k in range(KO):
                            nc.tensor.matmul(ps[:], lhsT=w_sb[:, k, bass.ts(m, 128)],
                                             rhs=x_sb[:, k], start=(k == 0),
                                             stop=(k == KO - 1))
                        if abias is None:
                            nc.scalar.activation(out_sb[:, m], ps[:], act)
                        else:
                            nc.scalar.activation(out_sb[:, m], ps[:], act, bias=abias)
                    return run

                pieces = [dma_piece]
                pieces += [mm_piece(wih_sb, m, xg_sb, AF.Identity, bias_sb[:, m:m + 1])
                           for m in range(MG)]
                pieces += [mm_piece(wc_sb, m, txc_sb, AF.Tanh, None) for m in range(MC)]
                return (x_sb, xg_sb, txc_sb), pieces

            cur, pieces0 = make_proj(0)
            for p in pieces0:
                p()

            prev_out = None
            for ch in range(NCH):
                x_sb, xg_sb, txc_sb = cur
                if ch + 1 < NCH:
                    nxt, pieces = make_proj(ch + 1)
                else:
                    nxt, pieces = None, []
                sched = {}
                for i, p in enumerate(pieces):
                    sched.setdefault(i * TCH // max(len(pieces), 1), []).append(p)

                ob = outp.tile([128, TCH + 1, MC, BC], BF16, tag="ob")
                if ch == 0:
                    nc.sync.dma_start(ob[:, 0], hT0.rearrange("(k p) b -> p k b", p=128))
                else:
                    nc.vector.tensor_copy(ob[:, 0], prev_out[:, TCH])

                for tl in range(TCH):
                    # off-chain precompute: d = c - txc_t
                    d = temp.tile([128, MC, BC], F32, tag="d")
                    nc.vector.tensor_sub(d[:], c_sb[:], txc_sb[:, :, tl])
                    # recurrent matmul, one PSUM bank per half so each half's
                    # elementwise tail starts as soon as its 16 matmuls finish
                    # (Tile dependencies are bank-granular).
                    pgh = [psum_r.tile([128, 2, 2, BC], F32, tag=f"pg{h}",
                                       name=f"pg{h}")
                           for h in range(2)]
                    for h in range(2):
                        for ci in range(2):
                            for g in range(2):          # f then o of cell chunk q
                                j = g * MC + 2 * h + ci
                                for k in range(KO):
                                    nc.tensor.matmul(
                                        pgh[h][:, g, ci],
                                        lhsT=whh_sb[:, k, bass.ts(j, 128)],
                                        rhs=ob[:, tl, k],
                                        start=(k == 0), stop=(k == KO - 1))
                    xg_r = xg_sb[:, :, tl].rearrange("p (g h c) b -> p h g c b", g=2, h=2)
                    for h in range(2):
                        cs = slice(2 * h, 2 * h + 2)
                        gp = temp.tile([128, 2, 2, BC], F32, tag="gp")
                        nc.vector.tensor_add(gp[:], pgh[h][:], xg_r[:, h])
                        g = temp.tile([128, 2, 2, BC], F32, tag="g")
                        nc.scalar.activation(g[:], gp[:], AF.Sigmoid)
                        # c' = f*d + txc
                        e = temp.tile([128, 2, BC], F32, tag="e")
                        nc.vector.tensor_mul(e[:], g[:, 0], d[:, cs])
                        nc.vector.tensor_add(c_sb[:, cs], e[:], txc_sb[:, cs, tl])
                        tch = temp.tile([128, 2, BC], F32, tag="tc")
                        nc.scalar.activation(tch[:], c_sb[:, cs], AF.Tanh)
                        # h = o*(tanh(c) - x) + x
                        u = temp.tile([128, 2, BC], F32, tag="u")
                        nc.vector.tensor_sub(u[:], tch[:], x_sb[:, cs, tl])
                        v = temp.tile([128, 2, BC], F32, tag="v")
                        nc.vector.tensor_mul(v[:], g[:, 1], u[:])
                        nc.vector.tensor_add(ob[:, tl + 1, cs], v[:], x_sb[:, cs, tl])
                    for p in sched.get(tl, []):
                        p()

                for k in range(MC):
                    nc.sync.dma_start(outD[k, :, bass.ts(ch, TCH), :], ob[:, 1:TCH + 1, k])
                prev_out = ob
                if nxt is not None:
                    cur = nxt

            nc.sync.dma_start(cfin[:], c_sb[:])

    _install_waitfix(nc)
    return nc


_NC = None


def _get_nc():
    global _NC
    if _NC is None:
        _NC = _build()
    return _NC


# ---------------------------------------------------------------------------
# Host-side shard / unshard
# ---------------------------------------------------------------------------

def _prep_core(x_rev_or_fwd, Wih, Whh, b, Wc, h0, c0, q):
    """Build the in_map for one core: batch quarter q of one direction."""
    bs = slice(q * BC, (q + 1) * BC)
    # x: [T,B,I] -> [NCH, I, TCH, BC]
    xq = x_rev_or_fwd[:, bs, :]                       # [T, BC, I]
    xq = np.ascontiguousarray(
        xq.reshape(NCH, TCH, BC, I).transpose(0, 3, 1, 2)
    ).astype(NP_BF16)
    return {
        "xT": xq,
        "wih": np.ascontiguousarray(Wih.T).astype(NP_BF16),
        "whh": np.ascontiguousarray(Whh.T).astype(NP_BF16),
        "wc": np.ascontiguousarray(Wc.T).astype(NP_BF16),
        "bias": np.ascontiguousarray(b.reshape(MG, 128)).astype(np.float32),
        "hT0": np.ascontiguousarray(h0[bs].T).astype(NP_BF16),
        "cT0": np.ascontiguousarray(c0[bs].T).astype(np.float32),
    }


def _assemble(results, reverse):
    """[4 cores] outD [MC,128,T,BC] bf16 -> outs [T,B,H] fp32, c_fin [B,H]."""
    outs = np.concatenate(
        [r["outD"].transpose(2, 0, 1, 3).reshape(T, H, BC) for r in results], axis=2
    ).astype(np.float32)                               # [T, H, B]
    outs = outs.transpose(0, 2, 1)                     # [T, B, H]
    cf = np.concatenate(
        [r["cfin"].transpose(1, 0, 2).reshape(H, BC) for r in results], axis=1
    ).astype(np.float32).T                             # [B, H]
    if reverse:
        outs = outs[::-1]
    return np.ascontiguousarray(outs), np.ascontiguousarray(cf)


def kernel(inputs, h_fwd, c_fwd, h_bwd, c_bwd,
           Wih_f, Whh_f, b_f, Wc_f,
           Wih_b, Whh_b, b_b, Wc_b,
           _trace=False):
    inputs = np.asarray(inputs, dtype=np.float32)
    x_rev = inputs[::-1]

    in_maps = []
    for q in range(4):
        in_maps.append(_prep_core(inputs, np.asarray(Wih_f), np.asarray(Whh_f),
                                  np.asarray(b_f), np.asarray(Wc_f),
                                  np.asarray(h_fwd), np.asarray(c_fwd), q))
    for q in range(4):
        in_maps.append(_prep_core(x_rev, np.asarray(Wih_b), np.asarray(Whh_b),
                                  np.asarray(b_b), np.asarray(Wc_b),
                                  np.asarray(h_bwd), np.asarray(c_bwd), q))

    nc = _get_nc()
    try:
        res = run_bass_kernel_spmd(nc, in_maps, core_ids=list(range(NCORES)),
                                   trace=_trace)
    except ModuleNotFoundError:
        res = run_bass_kernel_spmd(nc, in_maps, core_ids=list(range(NCORES)))

    outs_f, c_f = _assemble(res.results[0:4], reverse=False)
    outs_b, c_b = _assemble(res.results[4:8], reverse=True)
    outputs = np.concatenate([outs_f, outs_b], axis=-1)
    h_f = np.ascontiguousarray(outs_f[T - 1])
    h_b = np.ascontiguousarray(outs_b[0])

    if _trace:
        kernel.last_exec_time_ns = res.exec_time_ns
        kernel.last_trace = res.instructions_and_trace
    return outputs, h_f, c_f, h_b, c_b


# revision 14
# speedup vs baseline: 1.0525x; 1.0525x over previous
"""Bidirectional ReGU layer on 8 Trainium2 NeuronCores.

Problem: T=512, B=64, I=H=512.
  gates = sigmoid(x@Wih^T + h@Whh^T + b); f,o = split(gates)
  c = f*c + (1-f)*tanh(x@Wc^T);  h = o*tanh(c) + (1-o)*x
Forward scan + reverse scan, outputs concatenated on feature dim.

Sharding: core = (direction, batch quarter).  Cores 0-3 run the forward
scan on batch slices of 16, cores 4-7 the reverse scan (same SPMD program;
direction is carried entirely by the per-core input data: weights and a
time-reversed input tensor).  No collectives.

Layout: everything feature-on-partition ("transposed").  The recurrent
matmul is mapping-B: stationary Whh^T tiles (bf16 -> fast weight load),
moving operand h^T [128, 16].  Input projections are batched over 32
timesteps so their matmuls get N=512.  PSUM accumulates fp32; cell state c
stays fp32; h / staged projections are bf16 (validated: rel err ~3e-3).
"""

import json
import os

import numpy as np
import ml_dtypes

import concourse.bass as bass
import concourse.mybir as mybir
from concourse.tile import TileContext
from concourse.bass_utils import run_bass_kernel_spmd

AF = mybir.ActivationFunctionType
BF16 = mybir.dt.bfloat16
F32 = mybir.dt.float32
NP_BF16 = ml_dtypes.bfloat16

B, I, H = 64, 512, 512
NCORES = 8
BC = B // 4          # batch per core (4 quarters per direction)
TCH = 32             # timesteps per projection chunk
NCH = int(os.environ.get("REGU_NCH", "16"))  # chunks (16 => T=512)
T = NCH * TCH
KO = I // 128        # contraction k-tiles (4)
MG = 2 * H // 128    # gate feature tiles (8)
MC = H // 128        # cell feature tiles (4)

# ---------------------------------------------------------------------------
# Walrus workaround: this container's walrus rejects control instructions
# (Drain etc.) carrying more than ~2 semaphore waits.  Split excess waits
# onto single-wait EventSemaphore instructions on the same engine, same
# program position (semantics preserved).
# ---------------------------------------------------------------------------
_CTRL_OPS = {"Drain", "NoOp", "Halt", "EventSemaphore"}
_uid = [0]


def _split_ctrl_waits(bir_bytes, max_waits=1):
    bir = json.loads(bir_bytes)
    for fn in bir.get("functions", []):
        for blk in fn.get("blocks", []):
            out = []
            for inst in blk.get("instructions", []):
                si = inst.get("sync_info")
                waits = (si or {}).get("on_wait") or []
                cap = 1 if inst.get("opcode") in _CTRL_OPS else max_waits
                if si is not None and len(waits) > cap:
                    keep = waits[-cap:] if cap else []
                    for w in waits[: len(waits) - len(keep)]:
                        _uid[0] += 1
                        out.append({
                            "debug": inst.get("debug", 0),
                            "engine": inst["engine"],
                            "ins": [], "outs": [],
                            "name": f"I-wfix-{_uid[0]}",
                            "opcode": "EventSemaphore",
                            "sync_info": {"on_update": [], "on_wait": [w]},
                        })
                    inst["sync_info"] = {"on_update": si.get("on_update") or [], "on_wait": keep}
                out.append(inst)
            blk["instructions"] = out
    return json.dumps(bir).encode()


def _install_waitfix(nc):
    orig = nc.to_json_bytes
    nc.to_json_bytes = lambda: _split_ctrl_waits(orig())
    return nc


# ---------------------------------------------------------------------------
# Kernel graph (one SPMD program for all 8 cores)
# ---------------------------------------------------------------------------

def _build():
    nc = bass.Bass()

    # inputs (per core): time-major-chunked transposed x, transposed weights
    xT = nc.dram_tensor("xT", [NCH, I, TCH, BC], BF16, kind="ExternalInput")
    wih = nc.dram_tensor("wih", [I, 2 * H], BF16, kind="ExternalInput")   # Wih^T
    whh = nc.dram_tensor("whh", [H, 2 * H], BF16, kind="ExternalInput")   # Whh^T
    wc = nc.dram_tensor("wc", [I, H], BF16, kind="ExternalInput")         # Wc^T
    bias = nc.dram_tensor("bias", [MG, 128], F32, kind="ExternalInput")
    hT0 = nc.dram_tensor("hT0", [H, BC], BF16, kind="ExternalInput")
    cT0 = nc.dram_tensor("cT0", [H, BC], F32, kind="ExternalInput")

    # outputs: h for every step (bf16, feature-tiled), final c (fp32)
    outD = nc.dram_tensor("outD", [MC, 128, T, BC], BF16, kind="ExternalOutput")
    cfin = nc.dram_tensor("cfin", [128, MC, BC], F32, kind="ExternalOutput")

    with TileContext(nc) as tc:
        with (
            tc.tile_pool(name="const", bufs=1) as const,
            tc.tile_pool(name="stream", bufs=2) as stream,
            tc.tile_pool(name="outp", bufs=2) as outp,
            tc.tile_pool(name="temp", bufs=4) as temp,
            tc.tile_pool(name="psum_p", bufs=3, space="PSUM") as psum_p,
            tc.tile_pool(name="psum_r", bufs=4, space="PSUM") as psum_r,
        ):
            # ---- constants / state ----
            wih_sb = const.tile([128, KO, 2 * H], BF16, tag="wih")
            nc.sync.dma_start(wih_sb[:], wih.rearrange("(ko p) m -> p ko m", p=128))
            whh_sb = const.tile([128, KO, 2 * H], BF16, tag="whh")
            nc.sync.dma_start(whh_sb[:], whh.rearrange("(ko p) m -> p ko m", p=128))
            wc_sb = const.tile([128, KO, H], BF16, tag="wc")
            nc.sync.dma_start(wc_sb[:], wc.rearrange("(ko p) m -> p ko m", p=128))
            bias_sb = const.tile([128, MG], F32, tag="bias")
            nc.sync.dma_start(bias_sb[:], bias.rearrange("m p -> p m"))
            c_sb = const.tile([128, MC, BC], F32, tag="cstate")
            nc.sync.dma_start(c_sb[:], cT0.rearrange("(k p) b -> p k b", p=128))

            def make_proj(ch):
                """Chunk ch projections as 13 pieces, interleavable into the
                previous chunk's recurrence steps."""
                x_sb = stream.tile([128, KO, TCH, BC], BF16, tag="xch", name="xch")
                xg_sb = stream.tile([128, MG, TCH, BC], BF16, tag="xg", name="xg")
                txc_sb = stream.tile([128, MC, TCH, BC], BF16, tag="txc", name="txc")

                def dma_piece():
                    nc.sync.dma_start(
                        x_sb[:], xT[ch].rearrange("(ko p) t b -> p ko t b", p=128))

                def mm_piece(w_sb, m, out_sb, act, abias):
                    def run():
                        ps = psum_p.tile([128, TCH, BC], F32, tag="pp", name="pp")
                        for k in range(KO):
                            nc.tensor.matmul(ps[:], lhsT=w_sb[:, k, bass.ts(m, 128)],
                                             rhs=x_sb[:, k], start=(k == 0),
                                             stop=(k == KO - 1))
                        if abias is None:
                            nc.scalar.activation(out_sb[:, m], ps[:], act)
                        else:
                            nc.scalar.activation(out_sb[:, m], ps[:], act, bias=abias)
                    return run

                pieces = [dma_piece]
                pieces += [mm_piece(wih_sb, m, xg_sb, AF.Identity, bias_sb[:, m:m + 1])
                           for m in range(MG)]
                pieces += [mm_piece(wc_sb, m, txc_sb, AF.Tanh, None) for m in range(MC)]
                return (x_sb, xg_sb, txc_sb), pieces

            cur, pieces0 = make_proj(0)
            for p in pieces0:
                p()

            prev_out = None
            for ch in range(NCH):
                x_sb, xg_sb, txc_sb = cur
                if ch + 1 < NCH:
                    cur, pieces = make_proj(ch + 1)
                else:
                    pieces = []
                sched = {}
                for i, p in enumerate(pieces):
                    sched.setdefault(2 + i * (TCH - 4) // max(len(pieces), 1), []).append(p)

                # ---- recurrence over the chunk ----
                ob = outp.tile([128, TCH + 1, MC, BC], BF16, tag="ob")
                if ch == 0:
                    nc.sync.dma_start(ob[:, 0], hT0.rearrange("(k p) b -> p k b", p=128))
                else:
                    nc.vector.tensor_copy(ob[:, 0], prev_out[:, TCH])

                for tl in range(TCH):
                    # off-chain precompute: d = c - txc_t
                    d = temp.tile([128, MC, BC], F32, tag="d")
                    nc.vector.tensor_sub(d[:], c_sb[:], txc_sb[:, :, tl])
                    # recurrent matmul: gates^T += Whh^T @ h^T
                    pg = psum_r.tile([128, MG, BC], F32, tag="pg")
                    for j in range(MG):
                        for k in range(KO):
                            nc.tensor.matmul(pg[:, j], lhsT=whh_sb[:, k, bass.ts(j, 128)],
                                             rhs=ob[:, tl, k], start=(k == 0), stop=(k == KO - 1))
                    gp = temp.tile([128, MG, BC], F32, tag="gp")
                    nc.vector.tensor_add(gp[:], pg[:], xg_sb[:, :, tl])
                    g = temp.tile([128, MG, BC], F32, tag="g")
                    nc.scalar.activation(g[:], gp[:], AF.Sigmoid)
                    # c' = f*d + txc
                    e = temp.tile([128, MC, BC], F32, tag="e")
                    nc.vector.tensor_mul(e[:], g[:, 0:MC], d[:])
                    nc.vector.tensor_add(c_sb[:], e[:], txc_sb[:, :, tl])
                    tch = temp.tile([128, MC, BC], F32, tag="tc")
                    nc.scalar.activation(tch[:], c_sb[:], AF.Tanh)
                    # h = o*(tanh(c) - x) + x
                    u = temp.tile([128, MC, BC], F32, tag="u")
                    nc.vector.tensor_sub(u[:], tch[:], x_sb[:, :, tl])
                    v = temp.tile([128, MC, BC], F32, tag="v")
                    nc.vector.tensor_mul(v[:], g[:, MC:MG], u[:])
                    nc.vector.tensor_add(ob[:, tl + 1], v[:], x_sb[:, :, tl])
                    for p in sched.get(tl, []):
                        p()

                for k in range(MC):
                    nc.sync.dma_start(outD[k, :, bass.ts(ch, TCH), :], ob[:, 1:TCH + 1, k])
                prev_out = ob

            nc.sync.dma_start(cfin[:], c_sb[:])

    _install_waitfix(nc)
    return nc


_NC = None


def _get_nc():
    global _NC
    if _NC is None:
        _NC = _build()
    return _NC


# ---------------------------------------------------------------------------
# Host-side shard / unshard
# ---------------------------------------------------------------------------

def _prep_core(x_rev_or_fwd, Wih, Whh, b, Wc, h0, c0, q):
    """Build the in_map for one core: batch quarter q of one direction."""
    bs = slice(q * BC, (q + 1) * BC)
    # x: [T,B,I] -> [NCH, I, TCH, BC]
    xq = x_rev_or_fwd[:, bs, :]                       # [T, BC, I]
    xq = np.ascontiguousarray(
        xq.reshape(NCH, TCH, BC, I).transpose(0, 3, 1, 2)
    ).astype(NP_BF16)
    return {
        "xT": xq,
        "wih": np.ascontiguousarray(Wih.T).astype(NP_BF16),
        "whh": np.ascontiguousarray(Whh.T).astype(NP_BF16),
        "wc": np.ascontiguousarray(Wc.T).astype(NP_BF16),
        "bias": np.ascontiguousarray(b.reshape(MG, 128)).astype(np.float32),
        "hT0": np.ascontiguousarray(h0[bs].T).astype(NP_BF16),
        "cT0": np.ascontiguousarray(c0[bs].T).astype(np.float32),
    }


def _assemble(results, reverse):
    """[4 cores] outD [MC,128,T,BC] bf16 -> outs [T,B,H] fp32, c_fin [B,H]."""
    outs = np.concatenate(
        [r["outD"].transpose(2, 0, 1, 3).reshape(T, H, BC) for r in results], axis=2
    ).astype(np.float32)                               # [T, H, B]
    outs = outs.transpose(0, 2, 1)                     # [T, B, H]
    cf = np.concatenate(
        [r["cfin"].transpose(1, 0, 2).reshape(H, BC) for r in results], axis=1
    ).astype(np.float32).T                             # [B, H]
    if reverse:
        outs = outs[::-1]
    return np.ascontiguousarray(outs), np.ascontiguousarray(cf)


def kernel(inputs, h_fwd, c_fwd, h_bwd, c_bwd,
           Wih_f, Whh_f, b_f, Wc_f,
           Wih_b, Whh_b, b_b, Wc_b,
           _trace=False):
    inputs = np.asarray(inputs, dtype=np.float32)
    x_rev = inputs[::-1]

    in_maps = []
    for q in range(4):
        in_maps.append(_prep_core(inputs, np.asarray(Wih_f), np.asarray(Whh_f),
                                  np.asarray(b_f), np.asarray(Wc_f),
                                  np.asarray(h_fwd), np.asarray(c_fwd), q))
    for q in range(4):
        in_maps.append(_prep_core(x_rev, np.asarray(Wih_b), np.asarray(Whh_b),
                                  np.asarray(b_b), np.asarray(Wc_b),
                                  np.asarray(h_bwd), np.asarray(c_bwd), q))

    nc = _get_nc()
    try:
        res = run_bass_kernel_spmd(nc, in_maps, core_ids=list(range(NCORES)),
                                   trace=_trace)
    except ModuleNotFoundError:
        res = run_bass_kernel_spmd(nc, in_maps, core_ids=list(range(NCORES)))

    outs_f, c_f = _assemble(res.results[0:4], reverse=False)
    outs_b, c_b = _assemble(res.results[4:8], reverse=True)
    outputs = np.concatenate([outs_f, outs_b], axis=-1)
    h_f = np.ascontiguousarray(outs_f[T - 1])
    h_b = np.ascontiguousarray(outs_b[0])

    if _trace:
        kernel.last_exec_time_ns = res.exec_time_ns
        kernel.last_trace = res.instructions_and_trace
    return outputs, h_f, c_f, h_b, c_b
